# revision 1
# baseline (speedup 1.0000x reference)
"""Multi-head causal attention (B=2, S=2048, D=1024, H=16) on 8 TRN2 cores.

Sharding: tensor-parallel over heads. Core c owns heads {2c, 2c+1} and rows
[128c, 128c+128) of Wo. Each core computes its heads' attention and the
partial output projection; the host sums the 8 partials (the "all-reduce")
and adds the bias.

Device layout (all bf16 in SBUF, f32 PSUM accumulation):
  xT      [1024, 4096]  x transposed: xT[d, b*2048+s] = x[b,s,d]
  wq/wk/wv [1024, 128]  two heads' weights packed on columns
  wo      [128, 1024]   Wo rows for this core
  out_pT  [1024, 4096]  partial^T: out_pT[d, b*2048+s]

Per core:
  1. Q^T/K^T = (w.T @ xT) via PE, d-chunk outer so compute overlaps DMA-in;
     V computed directly in [t, k] layout (lhsT = xT s-block), padded with a
     leading ones column so the attention matmul also emits softmax
     denominators (in PSUM partition 0, readable by the custom-DVE fast
     reciprocal).
  2. Causal attention in scores^T orientation, quarter-major deferred AV:
     the scores/exp stream runs in kj order (exp on ACT; no max subtraction
     needed since |scores*scale| < ~1; triangular mask on the diagonal
     128x128 block), all exp tiles are kept in SBUF, and each 512-wide o^T
     quarter is accumulated as one dense burst of matmuls, then normalized
     (fast reciprocal + K=1 broadcast matmul) into OT.
  3. Partial projection out_pT = wo_rows.T @ OT, interleaved per 512-column
     chunk as PE filler inside the later heads' attention.
"""

import numpy as np
import ml_dtypes

B, S, D, H = 2, 2048, 1024, 16
HD = 64          # head dim
NCORES = 8
HL = H // NCORES  # local heads per core = 2
BS = B * S        # 4096
SCALE = float(D) ** -0.5

BF16 = ml_dtypes.bfloat16

_CACHE = {}


def _build_kernel():
    import concourse.mybir as mybir
    import concourse.tile as tile
    from concourse import bacc

    bf16 = mybir.dt.bfloat16
    f32 = mybir.dt.float32
    Exp = mybir.ActivationFunctionType.Exp

    nc = bacc.Bacc("TRN2", debug=False, enable_asserts=False)
    xT_d = nc.dram_tensor("xT", [D, BS], bf16, kind="ExternalInput").ap()
    wq_d = nc.dram_tensor("wq", [D, 128], bf16, kind="ExternalInput").ap()
    wk_d = nc.dram_tensor("wk", [D, 128], bf16, kind="ExternalInput").ap()
    wv_d = nc.dram_tensor("wv", [D, 128], bf16, kind="ExternalInput").ap()
    wo_d = nc.dram_tensor("wo", [128, D], bf16, kind="ExternalInput").ap()
    # consts: cols 0:128 = upper-tri mask (1 where col >= row), cols 128:192 =
    # 64x64 identity replicated in both partition halves.
    consts_d = nc.dram_tensor("consts", [128, 192], bf16, kind="ExternalInput").ap()
    out_d = nc.dram_tensor("out_pT", [D, BS], bf16, kind="ExternalOutput").ap()

    DC = D // 128   # 8 d-chunks
    NT = S // 128   # 16 key blocks per sequence

    with tile.TileContext(nc) as tc:
        with tc.tile_pool(name="persist", bufs=1) as pp:
            xT = pp.tile([128, DC, BS], bf16, tag="xT")
            qT = pp.tile([128, BS], bf16, tag="qT")
            kT = pp.tile([128, BS], bf16, tag="kT")
            # V in [t, k] layout, padded to 128 columns: col 0 = 1.0 (the
            # ones column makes the attention matmul emit softmax
            # denominators in PSUM partition 0, where the custom-DVE fast
            # reciprocal can read them), cols 1:64 = 0, cols 64:128 = V
            # block for s-block g (g = 16*b + t16) and local head j. The V
            # block starts at 64 so the 64 numerator rows of the PSUM output
            # sit at a size-aligned partition offset (a PSUM access rule).
            V_sb = pp.tile([128, BS // 128, HL, 128], bf16, tag="V")
            OT = pp.tile([128, BS], bf16, tag="OT")
            wq = pp.tile([128, DC, 128], bf16, tag="wq")
            wk = pp.tile([128, DC, 128], bf16, tag="wk")
            wv = pp.tile([128, DC, 128], bf16, tag="wv")
            wo = pp.tile([128, D], bf16, tag="wo")
            consts = pp.tile([128, 192], bf16, tag="consts")
            trimask = consts[:, 0:128]
            # 64x64 identity replicated in both partition halves, so the
            # V-transpose matmul sees lhsT and identity at the same base
            # partition for either local head.
            ident = consts[:, 128:192]
            ones64 = pp.tile([1, 64], f32, tag="ones64")

            # Constants. No gpsimd producers anywhere (a third producer engine
            # pushes consumers past the per-instruction sync-wait limit).
            nc.sync.dma_start(consts[:], consts_d[:])
            nc.vector.memset(ones64[:], 1.0)
            # Preheat the ACT exp table so the first real exp doesn't pay
            # the table-load latency mid-pipeline.
            warmup = pp.tile([1, 8], bf16, tag="warmup")
            nc.scalar.activation(warmup[:], consts[0:1, 0:8], Exp, scale=SCALE)
            nc.vector.memset(V_sb[:, :, :, 0:HD], 0.0)
            nc.vector.memset(V_sb[:, :, :, 0:1], 1.0)

            # DMA in. xT chunk 0 first (in quarters) so the first matmul
            # starts ASAP; the weight DMAs sit on the HW queues before the
            # remaining xT chunks, so phase-1's queue waits imply the weight
            # loads and later consumers (e.g. the first V matmul) don't need
            # an extra DMA wait (matmul carries at most 1 sync wait).
            xT_r = xT_d.rearrange("(o p) s -> p o s", p=128)
            for qq in range(4):
                nc.sync.dma_start(
                    xT[:, 0, 1024 * qq : 1024 * (qq + 1)],
                    xT_r[:, 0, 1024 * qq : 1024 * (qq + 1)],
                )
            for w_sb, w_dr in ((wq, wq_d), (wk, wk_d), (wv, wv_d)):
                nc.sync.dma_start(w_sb[:], w_dr.rearrange("(o p) c -> p o c", p=128))
            nc.sync.dma_start(wo[:], wo_d[:])
            for o in range(1, DC):
                for hh in range(2):
                    nc.sync.dma_start(
                        xT[:, o, 2048 * hh : 2048 * (hh + 1)],
                        xT_r[:, o, 2048 * hh : 2048 * (hh + 1)],
                    )

            # ---- Phase 1: Q^T / K^T projections, batch 0 only ----
            # Batch 1's projection chunks are deferred into the attention
            # loop as dense N=512 filler bursts (they're only needed from
            # the third head onward), shortening the serial prefix.
            with tc.tile_pool(name="ph1psum", bufs=8, space="PSUM") as ph1:
                for w_sb, dst in ((wq, qT), (wk, kT)):
                    ps = [ph1.tile([128, 512], f32, tag="ph1", name=f"ph1_{s}") for s in range(4)]
                    for o in range(DC):
                        for s in range(4):
                            nc.tensor.matmul(
                                ps[s][:],
                                lhsT=w_sb[:, o, :],
                                rhs=xT[:, o, 512 * s : 512 * (s + 1)],
                                start=(o == 0),
                                stop=(o == DC - 1),
                            )
                    for s in range(4):
                        nc.vector.tensor_copy(dst[:, 512 * s : 512 * (s + 1)], ps[s][:])

            # ---- Phase 2: V directly in [t, k] layout ----
            # lhsT = xT s-block (stationary), rhs = wv: psum[s, c] = V block
            # for both local heads side by side.
            # ---- Phase 2: V directly in [t, k] layout ----
            # lhsT = xT s-block (stationary), rhs = wv: psum[s, c] = V block
            # for both local heads side by side.
            def emit_v(vps, g, tag="pv"):
                pv = vps.tile([128, 128], f32, tag=tag, name="pv")
                for o in range(DC):
                    nc.tensor.matmul(
                        pv[:],
                        lhsT=xT[:, o, 128 * g : 128 * (g + 1)],
                        rhs=wv[:, o, :],
                        start=(o == 0),
                        stop=(o == DC - 1),
                    )
                nc.vector.tensor_copy(
                    V_sb[:, g, :, HD : 2 * HD],
                    pv[:].rearrange("p (j k) -> p j k", j=HL),
                )

            # All batch-0 V groups are deferred into the attention-phase
            # filler queue: group g is popped at key block g, first consumed
            # by the AV burst at key block 4*(ceil((g-3)/4))+4 > g.

            # ---- Phase 3: causal attention per (b, local head) ----
            # Quarter-major deferred AV: the scores/exp stream runs in kj
            # order (triple-buffered scores PSUM, so the PE stays ~3 blocks
            # ahead of ACT), with every exp tile kept alive in SBUF. The
            # o^T accumulation for s-quarter q (512 columns, one PSUM bank)
            # is emitted as one dense burst of 4q+4 matmuls right after
            # et(kj=4q+3) exists, then normalized; only two accumulator
            # banks are ever live. Dense AV bursts + a 3-deep scores
            # pipeline keep the PE gapless so the HAM clock-gate stays at
            # full clock. Batch 1's V projection and both batches' output
            # projections are interleaved as additional PE filler.
            with (
                tc.tile_pool(name="po", bufs=2, space="PSUM") as po_pool,
                tc.tile_pool(name="ps", bufs=2, space="PSUM") as ps_pool,
                tc.tile_pool(name="aux", bufs=2, space="PSUM") as aux_pool,
                tc.tile_pool(name="expp", bufs=26) as exp_pool,
                tc.tile_pool(name="recip", bufs=3) as rc_pool,
                tc.tile_pool(name="onum", bufs=3) as on_pool,
                tc.tile_pool(name="ph4out", bufs=2) as ph4o,
            ):
                def emit_block(b, j, kj, p0, p1):
                    qTh = qT[64 * j : 64 * (j + 1), S * b : S * (b + 1)]
                    kTh = kT[64 * j : 64 * (j + 1), S * b : S * (b + 1)]
                    s_lo = 128 * kj
                    w = p1 - p0
                    ps = ps_pool.tile([128, 1024], f32, tag="ps", name="ps")
                    for c0 in range(0, w, 512):
                        c1 = min(c0 + 512, w)
                        nc.tensor.matmul(
                            ps[:, c0:c1],
                            lhsT=kTh[:, s_lo : s_lo + 128],
                            rhs=qTh[:, p0 + c0 : p0 + c1],
                            start=True,
                            stop=True,
                        )
                    et = exp_pool.tile([128, 1024], bf16, tag="expT", name="et")
                    nc.scalar.activation(et[:, 0:w], ps[:, 0:w], Exp, scale=SCALE)
                    if p0 == s_lo:
                        # diagonal 128x128: keep only s' >= t
                        nc.vector.tensor_mul(et[:, 0:128], et[:, 0:128], trimask[:])
                    return et

                def emit_quarter(b, j, q, ets):
                    kj_last = 4 * q + 3
                    pq = po_pool.tile([128, 512], f32, tag="po", name="pq")
                    for k2 in range(kj_last + 1):
                        a0 = max(512 * q, 128 * k2)
                        a1 = 512 * (q + 1)
                        for p0, p1, et in ets[k2]:
                            if p0 <= a0 < p1:
                                break
                        else:
                            raise AssertionError("no piece")
                        nc.tensor.matmul(
                            pq[:, a0 - 512 * q : a1 - 512 * q],
                            lhsT=V_sb[:, NT * b + k2, j, :],
                            rhs=et[:, a0 - p0 : a1 - p0],
                            start=(k2 == 0),
                            stop=(k2 == kj_last),
                        )
                    return pq

                def emit_normalize(b, j, q, pq):
                    # o^T[k, s] / denom[s] for quarter q. Copy the numerator
                    # to SBUF (frees the po slot), fast-reciprocal the
                    # denominator row (PSUM partition 0), broadcast it
                    # across 64 partitions via a K=1 matmul, then a single
                    # one-PSUM-operand multiply into OT.
                    onum = on_pool.tile([64, 512], f32, tag="onum", name="onum")
                    nc.scalar.copy(onum[:], pq[HD : 2 * HD, :])
                    rc = rc_pool.tile([1, 512], f32, tag="rc", name="rc")
                    nc.vector.reciprocal_approx_fast(rc[:], pq[0:1, :])
                    pb = aux_pool.tile([64, 512], f32, tag="aux", name="pb")
                    nc.tensor.matmul(pb[:], lhsT=ones64[:], rhs=rc[:], start=True, stop=True)
                    nc.vector.tensor_mul(
                        OT[64 * j : 64 * (j + 1),
                           S * b + 512 * q : S * b + 512 * (q + 1)],
                        onum[:],
                        pb[:],
                    )

                out_r = out_d.rearrange("(o p) s -> p o s", p=128)

                def emit_ph4_nb(b, nb, rush=False):
                    # partial projection for 512 columns of batch b:
                    # out_pT[:, cols] = wo.T @ OT[:, cols]. rush=True (the
                    # kernel's last chunk): two half-stages with the casts
                    # alternating ACT/DVE so the tail isn't serialized on
                    # one engine.
                    for half in range(2 if rush else 1):
                        dcs = range(half * 4, half * 4 + 4) if rush else range(DC)
                        stage = ph4o.tile([128, DC, 512], bf16, tag="o4", name="o4")
                        for i, dc in enumerate(dcs):
                            pp4 = aux_pool.tile([128, 512], f32, tag="aux", name="pp4")
                            nc.tensor.matmul(
                                pp4[:],
                                lhsT=wo[:, 128 * dc : 128 * (dc + 1)],
                                rhs=OT[:, S * b + 512 * nb : S * b + 512 * (nb + 1)],
                                start=True,
                                stop=True,
                            )
                            if rush and i % 2 == 0:
                                nc.scalar.copy(stage[:, dc, :], pp4[:])
                            else:
                                nc.vector.tensor_copy(stage[:, dc, :], pp4[:])
                        rows = (slice(half * 4, half * 4 + 4) if rush
                                else slice(0, DC))
                        nc.sync.dma_start(
                            out_r[:, rows, S * b + 512 * nb : S * b + 512 * (nb + 1)],
                            stage[:, rows, :],
                        )

                def emit_qk1(w_sb, dst, sc):
                    # one batch-1 projection chunk: 8 accumulating matmuls
                    pk = aux_pool.tile([128, 512], f32, tag="aux", name="pk")
                    for o in range(DC):
                        nc.tensor.matmul(
                            pk[:],
                            lhsT=w_sb[:, o, :],
                            rhs=xT[:, o, 512 * sc : 512 * (sc + 1)],
                            start=(o == 0),
                            stop=(o == DC - 1),
                        )
                    nc.vector.tensor_copy(dst[:, 512 * sc : 512 * (sc + 1)], pk[:])

                filler_q = [("v", g) for g in range(0, BS // 256)]
                for sc in range(4, 8):
                    filler_q.append(("qk", wq, qT, sc))
                    filler_q.append(("qk", wk, kT, sc))
                filler_q += [("v", g) for g in range(BS // 256, BS // 128)]

                def emit_filler():
                    if not filler_q:
                        return
                    u = filler_q.pop(0)
                    if u[0] == "qk":
                        emit_qk1(u[1], u[2], u[3])
                    else:
                        emit_v(aux_pool, u[1], tag="aux")

                ph4_queue = []      # (b, nb) chunks awaiting emission
                ph4_state = None    # (b, nb, stage, next_dc)

                def emit_ph4_step():
                    # one dc-chunk of a pending output-projection unit
                    nonlocal ph4_state
                    if ph4_state is None:
                        if not ph4_queue:
                            return
                        b4, nb4 = ph4_queue.pop(0)
                        stage = ph4o.tile([128, DC, 512], bf16, tag="o4", name="o4")
                        ph4_state = (b4, nb4, stage, 0)
                    b4, nb4, stage, dc = ph4_state
                    pp4 = aux_pool.tile([128, 512], f32, tag="aux", name="pp4")
                    nc.tensor.matmul(
                        pp4[:],
                        lhsT=wo[:, 128 * dc : 128 * (dc + 1)],
                        rhs=OT[:, S * b4 + 512 * nb4 : S * b4 + 512 * (nb4 + 1)],
                        start=True,
                        stop=True,
                    )
                    nc.vector.tensor_copy(stage[:, dc, :], pp4[:])
                    if dc == DC - 1:
                        nc.sync.dma_start(
                            out_r[:, :, S * b4 + 512 * nb4 : S * b4 + 512 * (nb4 + 1)],
                            stage[:],
                        )
                        ph4_state = None
                    else:
                        ph4_state = (b4, nb4, stage, dc + 1)

                for bh in range(B * HL):
                    b, j = bh // HL, bh % HL
                    ets = {}
                    for kj in range(NT):
                        s_lo = 128 * kj
                        pieces = ([(s_lo, 1024), (1024, 2048)] if s_lo < 1024
                                  else [(s_lo, 2048)])
                        ets[kj] = [(p0, p1, emit_block(b, j, kj, p0, p1))
                                   for p0, p1 in pieces]
                        # quarter q's AV burst is deferred by one block so
                        # its final exp tile is ready when the burst reaches
                        # it (quarter 3 naturally gets this slack from the
                        # next head's first blocks)
                        if kj % 4 == 0 and kj > 0:
                            q = kj // 4 - 1
                            pq = emit_quarter(b, j, q, ets)
                            emit_normalize(b, j, q, pq)
                            if bh == 3:
                                ph4_queue.append((1, q))
                        if bh == 2 and kj % 4 == 0 and kj > 0:
                            # batch-0 output projection as bh2 filler (OT
                            # batch-0 columns completed during bh1),
                            # spread one dc-chunk per kj step below
                            ph4_queue.append((0, kj // 4 - 1))
                        if bh >= 2:
                            emit_ph4_step()
                            emit_ph4_step()
                        if bh <= 1:
                            # one filler unit per kj in bh0 (paced to the V
                            # deadlines), two per kj in bh1 to drain
                            emit_filler()
                            if bh == 1:
                                emit_filler()
                    q = 3
                    pq = emit_quarter(b, j, q, ets)
                    emit_normalize(b, j, q, pq)
                    if bh == 1:
                        # anything batch-1 still pending must land before bh2
                        while filler_q:
                            emit_filler()
                    if bh == 2:
                        ph4_queue.append((0, 3))
                    if bh == 3:
                        while ph4_queue or ph4_state is not None:
                            emit_ph4_step()
                        emit_ph4_nb(1, 3, rush=True)

    nc.compile()
    return nc


def get_nc():
    if "nc" not in _CACHE:
        _CACHE["nc"] = _build_kernel()
    return _CACHE["nc"]


def make_in_maps(x, Wq, Wk, Wv, Wo):
    """Host-side sharding: per-core input dict (numpy, bf16)."""
    x = np.asarray(x, np.float32)
    Wq = np.asarray(Wq, np.float32)
    Wk = np.asarray(Wk, np.float32)
    Wv = np.asarray(Wv, np.float32)
    Wo = np.asarray(Wo, np.float32)
    xT = np.ascontiguousarray(x.transpose(2, 0, 1).reshape(D, BS)).astype(BF16)
    in_maps = []
    for c in range(NCORES):
        h0 = HL * c

        def pack(W):
            # [HL, D, HD] -> [D, HL*HD]
            return np.ascontiguousarray(
                W[h0 : h0 + HL].transpose(1, 0, 2).reshape(D, HL * HD)
            ).astype(BF16)

        in_maps.append(
            {
                "xT": xT,
                "wq": pack(Wq),
                "wk": pack(Wk),
                "wv": pack(Wv),
                "wo": np.ascontiguousarray(Wo[128 * c : 128 * (c + 1), :]).astype(BF16),
                "consts": _make_consts(),
            }
        )
    return in_maps


def _make_consts():
    if "consts" not in _CACHE:
        tri = (np.arange(128)[None, :] >= np.arange(128)[:, None]).astype(np.float32)
        eye = np.eye(64, dtype=np.float32)
        c = np.zeros((128, 192), np.float32)
        c[:, 0:128] = tri
        c[0:64, 128:192] = eye
        c[64:128, 128:192] = eye
        _CACHE["consts"] = c.astype(BF16)
    return _CACHE["consts"]


def combine_partials(partials, bo):
    acc = np.zeros((D, BS), np.float32)
    for p in partials:
        acc += np.asarray(p, np.float32)
    out = acc.reshape(D, B, S).transpose(1, 2, 0) + np.asarray(bo, np.float32)[None, None, :]
    return np.ascontiguousarray(out.astype(np.float32))


def kernel(x, Wq, Wk, Wv, Wo, bo):
    from concourse.bass_utils import run_bass_kernel_spmd

    nc = get_nc()
    in_maps = make_in_maps(x, Wq, Wk, Wv, Wo)
    res = run_bass_kernel_spmd(nc, in_maps, core_ids=list(range(NCORES)))
    partials = [r["out_pT"] for r in res.results]
    return combine_partials(partials, bo)

